# revision 54
# baseline (speedup 1.0000x reference)
"""CTC prefix scorer on Trainium2 — Bass/Tile kernel, SPMD over 8 NeuronCores.

Math: the reference's 490-step lax.scan result is dead code (its output `r`
is only read at row start-1, which always comes from the LOGZERO prefix /
t=0 init), so the whole computation collapses to, per hypothesis h:

  log_psi[h, c] = log( sum_t w[t, h] * exp(x[b_h, t, c]) )

where w[t, h] = exp(rsum[t-1, h]) * [start <= t < xlen_b]  (normal labels)
          or  = exp(r_prev[t-1, 1, h]) * [...]             (c == last_ids[h])
with rsum = logaddexp(r_prev[:,0], r_prev[:,1]).  That is a (16 x T) @
(T x O) matmul per batch.  Frame masking folds into w (masked frames only
affect the BLANK/EOS output columns, which are overwritten anyway).  Final
output: scatter-select scored columns, EOS column = rsum[xlen-1], BLANK
column = LOGZERO, minus s_prev.

Sharding: core i <-> batch i (its 8 hypotheses).  x fully sharded on B.
"""

import numpy as np
from contextlib import ExitStack

import concourse.bass as bass
import concourse.tile as tile
from concourse import bacc, mybir
from concourse.bass import IndirectOffsetOnAxis
from concourse.bass_utils import run_bass_kernel_spmd
from concourse.tile_rust import add_dep_helper as _add_dep


def add_dep_helper(a, b, reason=""):
    """a depends on b; unwrap BassInstruction -> mybir.Instruction."""
    _add_dep(getattr(a, "ins", a), getattr(b, "ins", b), reason=reason)

F32 = mybir.dt.float32
F32R = mybir.dt.float32r
I32 = mybir.dt.int32
ACT = mybir.ActivationFunctionType
ALU = mybir.AluOpType

B, T, O = 8, 500, 10000
NH = 8                       # hypotheses per batch == per core
NCORES = 8
LOGZERO = -1e10
BLANK, EOS = 0, 2
SNUM = 200

NT = 512                     # N-tile width (one PSUM bank of f32)
N_TILES = [(c0, min(NT, O - c0)) for c0 in range(0, O, NT)]
K_CHUNKS = [(t0, min(128, T - t0)) for t0 in range(0, T, 128)]  # K over t<=499
NSCAT = (NH * SNUM + 127) // 128          # 13 indirect-scatter calls


def build_nc(start: int) -> bass.Bass:
    import os
    PHASE = os.environ.get("KDEBUG_PHASE", "full")
    nc = bacc.Bacc(None)
    x_d = nc.declare_dram_parameter("x", [T, O], F32, isOutput=False)
    rp_d = nc.declare_dram_parameter("rprev", [T, 2 * NH], F32, isOutput=False)
    sp_d = nc.declare_dram_parameter("sprev", [NH, O], F32, isOutput=False)
    li_d = nc.declare_dram_parameter("lastids", [NH, 1], I32, isOutput=False)
    mask_d = nc.declare_dram_parameter("smask", [NH, O], I32, isOutput=False)
    xl_d = nc.declare_dram_parameter("xlen", [128, 1], I32, isOutput=False)
    out_d = nc.declare_dram_parameter("out", [NH, O], F32, isOutput=True)

    with ExitStack() as ctx:
        tc = ctx.enter_context(tile.TileContext(nc))
        persist = ctx.enter_context(tc.tile_pool(name="persist", bufs=1))
        xpool = ctx.enter_context(tc.tile_pool(name="xp", bufs=8))
        psum = ctx.enter_context(tc.tile_pool(name="ps", bufs=4, space="PSUM"))
        epi = ctx.enter_context(tc.tile_pool(name="epi", bufs=6))

        # ---------------- xlen broadcast ------------------------------------
        xlb = persist.tile([128, 1], I32, tag="xlb")
        nc.sync.dma_start(out=xlb[:], in_=xl_d[:, :])
        xlb_f = persist.tile([128, 1], F32, tag="xlbf")
        nc.vector.tensor_copy(out=xlb_f[:], in_=xlb[:])

        # ---------------- eos score (DVE/ACT only, no PE) -------------------
        # eos[h] = rsum[xlen-1, h] = log(exp(r0[e,h]) + exp(r1[e,h]));
        # select row e = xlen-1 via a one-hot multiply + free-dim reduce.
        rpt = persist.tile([NH, 2 * T], F32, tag="rpt")
        nc.sync.dma_start(out=rpt[:, 0:T],
                          in_=rp_d[:, 0:NH].rearrange("t h -> h t"))
        nc.sync.dma_start(out=rpt[:, T:2 * T],
                          in_=rp_d[:, NH:2 * NH].rearrange("t h -> h t"))
        ept = persist.tile([NH, 2 * T], F32, tag="ept")
        nc.scalar.activation(ept[:], rpt[:], ACT.Exp)
        esum = persist.tile([NH, T], F32, tag="esum")
        nc.vector.tensor_tensor(out=esum[:], in0=ept[:, 0:T],
                                in1=ept[:, T:2 * T], op=ALU.add)
        iot_i = persist.tile([NH, T], I32, tag="ioti")
        nc.gpsimd.iota(iot_i[:], pattern=[[1, T]], base=0, channel_multiplier=0)
        iot_f = persist.tile([NH, T], F32, tag="iotf")
        nc.vector.tensor_copy(out=iot_f[:], in_=iot_i[:])
        ohm = persist.tile([NH, T], F32, tag="ohm")
        nc.vector.tensor_scalar(out=ohm[:], in0=iot_f[:], scalar1=1.0,
                                scalar2=xlb_f[0:NH, :1], op0=ALU.add,
                                op1=ALU.is_equal)  # (t+1)==xlen
        emsk = persist.tile([NH, T], F32, tag="emsk")
        nc.vector.tensor_tensor(out=emsk[:], in0=esum[:], in1=ohm[:],
                                op=ALU.mult)
        esel = persist.tile([NH, 1], F32, tag="esel")
        nc.vector.reduce_sum(out=esel[:], in_=emsk[:],
                             axis=mybir.AxisListType.X)
        eos_sb = persist.tile([NH, 1], F32, tag="eos")
        nc.scalar.activation(eos_sb[:], esel[:], ACT.Ln)

        # ---------------- lhsT weights --------------------------------------
        # lhsT row t (global) <- r_prev[t-1]; chunk k covers t in [128k,128k+128)
        lhsTs = []
        for k, (t0, _) in enumerate(K_CHUNKS):
            a, b = max(t0, 1), min(t0 + 128, T)
            pa, pb = a - t0, b - t0
            # full-128-partition ops only (SBUF compute APs must start at
            # partition 0): unloaded rows hold exp(0)=1 etc., neutralized by
            # the wm/oh masks below (always 0 there).
            e_t = persist.tile([128, 2 * NH], F32, tag=f"e{k}")
            nc.gpsimd.memset(e_t[:], 0.0)
            nc.sync.dma_start(out=e_t[pa:pb, :], in_=rp_d[a - 1:b - 1, :])
            nc.scalar.activation(e_t[:], e_t[:], ACT.Exp)
            sum_t = persist.tile([128, NH], F32, tag=f"sum{k}")
            nc.vector.tensor_tensor(out=sum_t[:], in0=e_t[:, 0:NH],
                                    in1=e_t[:, NH:2 * NH], op=ALU.add)

            io_t = persist.tile([128, 1], I32, tag=f"io{k}")
            nc.gpsimd.iota(io_t[:], pattern=[[0, 1]], base=t0, channel_multiplier=1)
            io_f = persist.tile([128, 1], F32, tag=f"iof{k}")
            nc.vector.tensor_copy(out=io_f[:], in_=io_t[:])
            ge_t = persist.tile([128, 1], F32, tag=f"ge{k}")
            nc.vector.tensor_scalar(out=ge_t[:], in0=io_f[:], scalar1=float(start),
                                    scalar2=None, op0=ALU.is_ge)
            lt_t = persist.tile([128, 1], F32, tag=f"lt{k}")
            nc.vector.tensor_scalar(out=lt_t[:], in0=io_f[:], scalar1=xlb_f[:, :1],
                                    scalar2=None, op0=ALU.is_lt)
            wm_t = persist.tile([128, 1], F32, tag=f"wm{k}")
            nc.vector.tensor_tensor(out=wm_t[:], in0=ge_t[:], in1=lt_t[:],
                                    op=ALU.mult)

            # w1 half lives at col 32 so the matmul output lands at PSUM
            # partition 32 (hardware requires partition starts in {0,32,64,96})
            # cols 8:32 are padding (psum partitions 8..31 unread); fill with
            # wm so their psum sums stay positive and Ln of the full tile is
            # finite (keeps the simulator's finiteness checks enabled).
            w_t = persist.tile([128, 32 + NH], F32R, tag=f"w{k}")
            nc.vector.tensor_scalar(out=w_t[:, NH:32],
                                    in0=wm_t[:, :1].to_broadcast([128, 32 - NH]),
                                    scalar1=1.0, scalar2=None, op0=ALU.mult)
            nc.vector.tensor_scalar(out=w_t[:, 0:NH], in0=sum_t[:], scalar1=wm_t[:, :1],
                                    scalar2=None, op0=ALU.mult)
            nc.vector.tensor_scalar(out=w_t[:, 32:32 + NH], in0=e_t[:, NH:2 * NH],
                                    scalar1=wm_t[:, :1], scalar2=None, op0=ALU.mult)
            lhsTs.append(w_t)

        # ---------------- shared epilogue constants -------------------------
        iotac_i = persist.tile([NH, NT], I32, tag="iotaci")
        nc.gpsimd.iota(iotac_i[:], pattern=[[1, NT]], base=0, channel_multiplier=0)
        iotac = persist.tile([NH, NT], F32, tag="iotac")
        nc.vector.tensor_copy(out=iotac[:], in_=iotac_i[:])
        li_t = persist.tile([NH, 1], I32, tag="li")
        nc.sync.dma_start(out=li_t[:], in_=li_d[:, :])
        li_f = persist.tile([NH, 1], F32, tag="lif")
        nc.vector.tensor_copy(out=li_f[:], in_=li_t[:])

        # ---------------- main loop: groups of 4 N-tiles ---------------------
        # Grouping batches the ACT work (16 Exp, then 4 Ln) so the activation
        # table isn't reloaded per tile (1.3us per Exp<->Ln switch).
        GRP = 4
        for g0 in range(0, len(N_TILES), GRP):
            group = list(enumerate(N_TILES))[g0:g0 + GRP]
            accs = []
            for j, (c0, N) in group:
                acc = psum.tile([32 + NH, NT], F32, tag="acc")
                for k, (t0, K) in enumerate(K_CHUNKS):
                    xraw = xpool.tile([128, NT], F32, tag="xraw")
                    nc.sync.dma_start(out=xraw[:K, :N],
                                      in_=x_d[t0:t0 + K, c0:c0 + N])
                    xt = xpool.tile([128, NT], F32R, tag="xt")
                    nc.scalar.activation(xt[:K, :N], xraw[:K, :N], ACT.Exp)
                    nc.tensor.matmul(out=acc[:, :N], lhsT=lhsTs[k][:K, :],
                                     rhs=xt[:K, :N],
                                     start=(k == 0), stop=(k == len(K_CHUNKS) - 1))
                accs.append(acc)

            for (j, (c0, N)), acc in zip(group, accs):
                # one Ln over partitions 0..39: rows 0:8 = la0, 32:40 = la1
                la = epi.tile([32 + NH, NT], F32, tag="la")
                nc.scalar.activation(la[:, :N], acc[:, :N], ACT.Ln)

                lastc0 = epi.tile([NH, 1], F32, tag="lastc0")
                nc.gpsimd.tensor_scalar(out=lastc0[:], in0=li_f[:],
                                        scalar1=float(c0), scalar2=None,
                                        op0=ALU.subtract)
                # hit mask written at base partition 32 so mask and data of
                # copy_predicated share a base partition (hardware requires
                # equal SBUF input bases for 2-input DVE ops)
                hit = epi.tile([32 + NH, NT], I32, tag="hit")
                nc.gpsimd.tensor_scalar(out=hit[32:32 + NH, :N], in0=iotac[:, :N],
                                        scalar1=lastc0[:, :1], scalar2=None,
                                        op0=ALU.is_equal)
                # merge the hit column into la0 in place
                nc.vector.copy_predicated(out=la[0:NH, :N],
                                          mask=hit[32:32 + NH, :N],
                                          data=la[32:32 + NH, :N])

                sm = epi.tile([NH, NT], I32, tag="sm")
                nc.sync.dma_start(out=sm[:, :N], in_=mask_d[:, c0:c0 + N])
                spv = epi.tile([NH, NT], F32, tag="spv")
                nc.sync.dma_start(out=spv[:, :N], in_=sp_d[:, c0:c0 + N])

                val2 = epi.tile([NH, NT], F32, tag="val2")
                nc.gpsimd.tensor_tensor(out=val2[:, :N], in0=la[0:NH, :N],
                                        in1=spv[:, :N], op=ALU.subtract)
                fin = epi.tile([NH, NT], F32, tag="fin")
                nc.gpsimd.tensor_scalar(out=fin[:, :N], in0=spv[:, :N],
                                        scalar1=-1.0, scalar2=LOGZERO,
                                        op0=ALU.mult, op1=ALU.add)
                nc.vector.copy_predicated(out=fin[:, :N], mask=sm[:, :N],
                                          data=val2[:, :N])
                if j == 0:
                    # EOS col: eos - s_prev (BLANK col already LOGZERO - s_prev
                    # since smask[BLANK]=0 is forced host-side)
                    nc.vector.tensor_tensor(out=fin[:, EOS:EOS + 1],
                                            in0=eos_sb[:],
                                            in1=spv[:, EOS:EOS + 1],
                                            op=ALU.subtract)
                nc.sync.dma_start(out=out_d[:, c0:c0 + N], in_=fin[:, :N])

    nc.compile()
    return nc


def make_in_maps(x, r_prev, s_prev, xlens, last_ids, scoring_ids):
    """Per-core input maps: core i owns batch i / hypotheses [8i, 8i+8)."""
    in_maps = []
    for i in range(NCORES):
        hs = slice(i * NH, (i + 1) * NH)
        sids = np.ascontiguousarray(scoring_ids[hs]).astype(np.int64)  # (8,200)
        smask = np.zeros((NH, O), np.int32)
        np.put_along_axis(smask, sids, 1, axis=1)
        smask[:, BLANK] = 0  # BLANK output column is always LOGZERO - s_prev
        in_maps.append({
            "x": np.ascontiguousarray(x[i]).astype(np.float32),
            "rprev": np.ascontiguousarray(r_prev[:, :, hs]).reshape(T, 2 * NH).astype(np.float32),
            "sprev": np.ascontiguousarray(s_prev[hs]).astype(np.float32),
            "lastids": np.ascontiguousarray(last_ids[hs]).astype(np.int32)[:, None],
            "smask": smask,
            "xlen": np.full((128, 1), int(xlens[i]), np.int32),
        })
    return in_maps


_NC_CACHE: dict[int, bass.Bass] = {}


def kernel(x, r_prev, s_prev, xlens, last_ids, scoring_ids, output_length,
           _trace=False):
    x = np.asarray(x)
    r_prev = np.asarray(r_prev)
    s_prev = np.asarray(s_prev)
    xlens = np.asarray(xlens)
    last_ids = np.asarray(last_ids)
    scoring_ids = np.asarray(scoring_ids)
    start = max(int(output_length), 1)
    # output_length == 0 adds an extra x_[0,0] term; inputs here always have
    # output_length >= 1, which this kernel implements.
    assert int(output_length) >= 1, "output_length==0 path not implemented"

    if start not in _NC_CACHE:
        _NC_CACHE[start] = build_nc(start)
    nc = _NC_CACHE[start]

    in_maps = make_in_maps(x, r_prev, s_prev, xlens, last_ids, scoring_ids)
    res = run_bass_kernel_spmd(nc, in_maps, core_ids=list(range(NCORES)),
                               trace=_trace)
    out = np.concatenate([res.results[i]["out"] for i in range(NCORES)], axis=0)
    kernel.last_exec_time_ns = res.exec_time_ns
    kernel.last_results = res
    return out.astype(np.float32)


# revision 56
# speedup vs baseline: 1.5053x; 1.5053x over previous
"""CTC prefix scorer on Trainium2 — Bass/Tile kernel, SPMD over 8 NeuronCores.

Math: the reference's 490-step lax.scan result is dead code (its output `r`
is only read at row start-1, which always comes from the LOGZERO prefix /
t=0 init), so the whole computation collapses to, per hypothesis h:

  log_psi[h, c] = log( sum_t w[t, h] * exp(x[b_h, t, c]) )

where w[t, h] = exp(rsum[t-1, h]) * [start <= t < xlen_b]  (normal labels)
          or  = exp(r_prev[t-1, 1, h]) * [...]             (c == last_ids[h])
with rsum = logaddexp(r_prev[:,0], r_prev[:,1]).  That is a (16 x T) @
(T x O) matmul per batch.  Frame masking folds into w (masked frames only
affect the BLANK/EOS output columns, which are overwritten anyway).  Final
output: scatter-select scored columns, EOS column = rsum[xlen-1], BLANK
column = LOGZERO, minus s_prev.

Sharding: core i <-> batch i (its 8 hypotheses).  x fully sharded on B.
"""

import numpy as np
from contextlib import ExitStack

import concourse.bass as bass
import concourse.tile as tile
from concourse import bacc, mybir
from concourse.bass import IndirectOffsetOnAxis
from concourse.bass_utils import run_bass_kernel_spmd
from concourse.tile_rust import add_dep_helper as _add_dep


def add_dep_helper(a, b, sync=True, reason=""):
    """a depends on b; unwrap BassInstruction -> mybir.Instruction."""
    _add_dep(getattr(a, "ins", a), getattr(b, "ins", b), sync=sync, reason=reason)

F32 = mybir.dt.float32
F32R = mybir.dt.float32r
I32 = mybir.dt.int32
ACT = mybir.ActivationFunctionType
ALU = mybir.AluOpType

B, T, O = 8, 500, 10000
NH = 8                       # hypotheses per batch == per core
NCORES = 8
LOGZERO = -1e10
BLANK, EOS = 0, 2
SNUM = 200

NT = 512                     # N-tile width (one PSUM bank of f32)
N_TILES = [(c0, min(NT, O - c0)) for c0 in range(0, O, NT)]
K_CHUNKS = [(t0, min(128, T - t0)) for t0 in range(0, T, 128)]  # K over t<=499
NSCAT = (NH * SNUM + 127) // 128          # 13 indirect-scatter calls


def build_nc(start: int) -> bass.Bass:
    import os
    PHASE = os.environ.get("KDEBUG_PHASE", "full")
    nc = bacc.Bacc(None)
    x_d = nc.declare_dram_parameter("x", [T, O], F32, isOutput=False)
    rp_d = nc.declare_dram_parameter("rprev", [T, 2 * NH], F32, isOutput=False)
    sp_d = nc.declare_dram_parameter("sprev", [NH, O], F32, isOutput=False)
    li_d = nc.declare_dram_parameter("lastids", [NH, 1], I32, isOutput=False)
    mask_d = nc.declare_dram_parameter("smask", [NH, O], I32, isOutput=False)
    xl_d = nc.declare_dram_parameter("xlen", [128, 1], I32, isOutput=False)
    out_d = nc.declare_dram_parameter("out", [NH, O], F32, isOutput=True)

    with ExitStack() as ctx:
        tc = ctx.enter_context(tile.TileContext(nc))
        persist = ctx.enter_context(tc.tile_pool(name="persist", bufs=1))
        xpool = ctx.enter_context(tc.tile_pool(name="xp", bufs=8))
        psum = ctx.enter_context(tc.tile_pool(name="ps", bufs=4, space="PSUM"))
        epi = ctx.enter_context(tc.tile_pool(name="epi", bufs=6))

        # ---------------- xlen broadcast ------------------------------------
        xlb = persist.tile([128, 1], I32, tag="xlb")
        nc.sync.dma_start(out=xlb[:], in_=xl_d[:, :])
        xlb_f = persist.tile([128, 1], F32, tag="xlbf")
        nc.vector.tensor_copy(out=xlb_f[:], in_=xlb[:])

        # ---------------- eos score (DVE/ACT only, no PE) -------------------
        # eos[h] = rsum[xlen-1, h] = log(exp(r0[e,h]) + exp(r1[e,h]));
        # select row e = xlen-1 via a one-hot multiply + free-dim reduce.
        rpt = persist.tile([NH, 2 * T], F32, tag="rpt")
        nc.sync.dma_start(out=rpt[:, 0:T],
                          in_=rp_d[:, 0:NH].rearrange("t h -> h t"))
        nc.sync.dma_start(out=rpt[:, T:2 * T],
                          in_=rp_d[:, NH:2 * NH].rearrange("t h -> h t"))
        ept = persist.tile([NH, 2 * T], F32, tag="ept")
        nc.scalar.activation(ept[:], rpt[:], ACT.Exp)
        esum = persist.tile([NH, T], F32, tag="esum")
        nc.vector.tensor_tensor(out=esum[:], in0=ept[:, 0:T],
                                in1=ept[:, T:2 * T], op=ALU.add)
        iot_i = persist.tile([NH, T], I32, tag="ioti")
        nc.gpsimd.iota(iot_i[:], pattern=[[1, T]], base=0, channel_multiplier=0)
        iot_f = persist.tile([NH, T], F32, tag="iotf")
        nc.vector.tensor_copy(out=iot_f[:], in_=iot_i[:])
        ohm = persist.tile([NH, T], F32, tag="ohm")
        nc.vector.tensor_scalar(out=ohm[:], in0=iot_f[:], scalar1=1.0,
                                scalar2=xlb_f[0:NH, :1], op0=ALU.add,
                                op1=ALU.is_equal)  # (t+1)==xlen
        emsk = persist.tile([NH, T], F32, tag="emsk")
        nc.vector.tensor_tensor(out=emsk[:], in0=esum[:], in1=ohm[:],
                                op=ALU.mult)
        esel = persist.tile([NH, 1], F32, tag="esel")
        nc.vector.reduce_sum(out=esel[:], in_=emsk[:],
                             axis=mybir.AxisListType.X)
        eos_sb = persist.tile([NH, 1], F32, tag="eos")
        nc.scalar.activation(eos_sb[:], esel[:], ACT.Ln)

        # ---------------- lhsT weights --------------------------------------
        # lhsT row t (global) <- r_prev[t-1]; chunk k covers t in [128k,128k+128)
        lhsTs = []
        for k, (t0, _) in enumerate(K_CHUNKS):
            a, b = max(t0, 1), min(t0 + 128, T)
            pa, pb = a - t0, b - t0
            # full-128-partition ops only (SBUF compute APs must start at
            # partition 0): unloaded rows hold exp(0)=1 etc., neutralized by
            # the wm/oh masks below (always 0 there).
            e_t = persist.tile([128, 2 * NH], F32, tag=f"e{k}")
            nc.gpsimd.memset(e_t[:], 0.0)
            nc.sync.dma_start(out=e_t[pa:pb, :], in_=rp_d[a - 1:b - 1, :])
            nc.scalar.activation(e_t[:], e_t[:], ACT.Exp)
            sum_t = persist.tile([128, NH], F32, tag=f"sum{k}")
            nc.vector.tensor_tensor(out=sum_t[:], in0=e_t[:, 0:NH],
                                    in1=e_t[:, NH:2 * NH], op=ALU.add)

            io_t = persist.tile([128, 1], I32, tag=f"io{k}")
            nc.gpsimd.iota(io_t[:], pattern=[[0, 1]], base=t0, channel_multiplier=1)
            io_f = persist.tile([128, 1], F32, tag=f"iof{k}")
            nc.vector.tensor_copy(out=io_f[:], in_=io_t[:])
            ge_t = persist.tile([128, 1], F32, tag=f"ge{k}")
            nc.vector.tensor_scalar(out=ge_t[:], in0=io_f[:], scalar1=float(start),
                                    scalar2=None, op0=ALU.is_ge)
            lt_t = persist.tile([128, 1], F32, tag=f"lt{k}")
            nc.vector.tensor_scalar(out=lt_t[:], in0=io_f[:], scalar1=xlb_f[:, :1],
                                    scalar2=None, op0=ALU.is_lt)
            wm_t = persist.tile([128, 1], F32, tag=f"wm{k}")
            nc.vector.tensor_tensor(out=wm_t[:], in0=ge_t[:], in1=lt_t[:],
                                    op=ALU.mult)

            # w1 half lives at col 32 so the matmul output lands at PSUM
            # partition 32 (hardware requires partition starts in {0,32,64,96})
            # cols 8:32 are padding (psum partitions 8..31 unread); fill with
            # wm so their psum sums stay positive and Ln of the full tile is
            # finite (keeps the simulator's finiteness checks enabled).
            w_t = persist.tile([128, 32 + NH], F32R, tag=f"w{k}")
            nc.vector.tensor_scalar(out=w_t[:, NH:32],
                                    in0=wm_t[:, :1].to_broadcast([128, 32 - NH]),
                                    scalar1=1.0, scalar2=None, op0=ALU.mult)
            nc.vector.tensor_scalar(out=w_t[:, 0:NH], in0=sum_t[:], scalar1=wm_t[:, :1],
                                    scalar2=None, op0=ALU.mult)
            nc.vector.tensor_scalar(out=w_t[:, 32:32 + NH], in0=e_t[:, NH:2 * NH],
                                    scalar1=wm_t[:, :1], scalar2=None, op0=ALU.mult)
            lhsTs.append(w_t)

        # ---------------- shared epilogue constants -------------------------
        iotac_i = persist.tile([NH, NT], I32, tag="iotaci")
        nc.gpsimd.iota(iotac_i[:], pattern=[[1, NT]], base=0, channel_multiplier=0)
        iotac = persist.tile([NH, NT], F32, tag="iotac")
        nc.vector.tensor_copy(out=iotac[:], in_=iotac_i[:])
        li_t = persist.tile([NH, 1], I32, tag="li")
        nc.sync.dma_start(out=li_t[:], in_=li_d[:, :])
        li_f = persist.tile([NH, 1], F32, tag="lif")
        nc.vector.tensor_copy(out=li_f[:], in_=li_t[:])

        # ---------------- main loop: groups of 4 N-tiles ---------------------
        # Grouping batches the ACT work (16 Exp, then 4 Ln) so the activation
        # table isn't reloaded per tile (1.3us per Exp<->Ln switch).
        GRP = 4
        prev_ln_insts = []
        for g0 in range(0, len(N_TILES), GRP):
            group = list(enumerate(N_TILES))[g0:g0 + GRP]
            accs = []
            exp_insts, ln_insts = [], []
            for j, (c0, N) in group:
                acc = psum.tile([32 + NH, NT], F32, tag="acc")
                for k, (t0, K) in enumerate(K_CHUNKS):
                    xraw = xpool.tile([128, NT], F32, tag="xraw")
                    nc.sync.dma_start(out=xraw[:K, :N],
                                      in_=x_d[t0:t0 + K, c0:c0 + N])
                    xt = xpool.tile([128, NT], F32R, tag="xt")
                    ei = nc.scalar.activation(xt[:K, :N], xraw[:K, :N], ACT.Exp)
                    exp_insts.append(ei)
                    nc.tensor.matmul(out=acc[:, :N], lhsT=lhsTs[k][:K, :],
                                     rhs=xt[:K, :N],
                                     start=(k == 0), stop=(k == len(K_CHUNKS) - 1))
                accs.append(acc)

            # keep the ACT stream batched (16 Exp then 4 Ln per group): order
            # this group's Exps after the previous group's Lns, schedule-only.
            for ei in exp_insts:
                for li in prev_ln_insts:
                    add_dep_helper(ei, li, sync=False,
                                   reason="batch ACT table usage")

            for (j, (c0, N)), acc in zip(group, accs):
                # one Ln over partitions 0..39: rows 0:8 = la0, 32:40 = la1
                la = epi.tile([32 + NH, NT], F32, tag="la")
                li_ = nc.scalar.activation(la[:, :N], acc[:, :N], ACT.Ln)
                ln_insts.append(li_)

                lastc0 = epi.tile([NH, 1], F32, tag="lastc0")
                nc.vector.tensor_scalar(out=lastc0[:], in0=li_f[:],
                                        scalar1=float(c0), scalar2=None,
                                        op0=ALU.subtract)
                # hit mask written at base partition 32 so mask and data of
                # copy_predicated share a base partition (hardware requires
                # equal SBUF input bases for 2-input DVE ops)
                hit = epi.tile([32 + NH, NT], I32, tag="hit")
                nc.vector.tensor_scalar(out=hit[32:32 + NH, :N], in0=iotac[:, :N],
                                        scalar1=lastc0[:, :1], scalar2=None,
                                        op0=ALU.is_equal)
                # merge the hit column into la0 in place
                nc.vector.copy_predicated(out=la[0:NH, :N],
                                          mask=hit[32:32 + NH, :N],
                                          data=la[32:32 + NH, :N])

                sm = epi.tile([NH, NT], I32, tag="sm")
                nc.sync.dma_start(out=sm[:, :N], in_=mask_d[:, c0:c0 + N])
                spv = epi.tile([NH, NT], F32, tag="spv")
                nc.sync.dma_start(out=spv[:, :N], in_=sp_d[:, c0:c0 + N])

                val2 = epi.tile([NH, NT], F32, tag="val2")
                nc.vector.tensor_tensor(out=val2[:, :N], in0=la[0:NH, :N],
                                        in1=spv[:, :N], op=ALU.subtract)
                fin = epi.tile([NH, NT], F32, tag="fin")
                nc.vector.tensor_scalar(out=fin[:, :N], in0=spv[:, :N],
                                        scalar1=-1.0, scalar2=LOGZERO,
                                        op0=ALU.mult, op1=ALU.add)
                nc.vector.copy_predicated(out=fin[:, :N], mask=sm[:, :N],
                                          data=val2[:, :N])
                if j == 0:
                    # EOS col: eos - s_prev (BLANK col already LOGZERO - s_prev
                    # since smask[BLANK]=0 is forced host-side)
                    nc.vector.tensor_tensor(out=fin[:, EOS:EOS + 1],
                                            in0=eos_sb[:],
                                            in1=spv[:, EOS:EOS + 1],
                                            op=ALU.subtract)
                nc.sync.dma_start(out=out_d[:, c0:c0 + N], in_=fin[:, :N])
            prev_ln_insts = ln_insts

    nc.compile()
    return nc


def make_in_maps(x, r_prev, s_prev, xlens, last_ids, scoring_ids):
    """Per-core input maps: core i owns batch i / hypotheses [8i, 8i+8)."""
    in_maps = []
    for i in range(NCORES):
        hs = slice(i * NH, (i + 1) * NH)
        sids = np.ascontiguousarray(scoring_ids[hs]).astype(np.int64)  # (8,200)
        smask = np.zeros((NH, O), np.int32)
        np.put_along_axis(smask, sids, 1, axis=1)
        smask[:, BLANK] = 0  # BLANK output column is always LOGZERO - s_prev
        in_maps.append({
            "x": np.ascontiguousarray(x[i]).astype(np.float32),
            "rprev": np.ascontiguousarray(r_prev[:, :, hs]).reshape(T, 2 * NH).astype(np.float32),
            "sprev": np.ascontiguousarray(s_prev[hs]).astype(np.float32),
            "lastids": np.ascontiguousarray(last_ids[hs]).astype(np.int32)[:, None],
            "smask": smask,
            "xlen": np.full((128, 1), int(xlens[i]), np.int32),
        })
    return in_maps


_NC_CACHE: dict[int, bass.Bass] = {}


def kernel(x, r_prev, s_prev, xlens, last_ids, scoring_ids, output_length,
           _trace=False):
    x = np.asarray(x)
    r_prev = np.asarray(r_prev)
    s_prev = np.asarray(s_prev)
    xlens = np.asarray(xlens)
    last_ids = np.asarray(last_ids)
    scoring_ids = np.asarray(scoring_ids)
    start = max(int(output_length), 1)
    # output_length == 0 adds an extra x_[0,0] term; inputs here always have
    # output_length >= 1, which this kernel implements.
    assert int(output_length) >= 1, "output_length==0 path not implemented"

    if start not in _NC_CACHE:
        _NC_CACHE[start] = build_nc(start)
    nc = _NC_CACHE[start]

    in_maps = make_in_maps(x, r_prev, s_prev, xlens, last_ids, scoring_ids)
    res = run_bass_kernel_spmd(nc, in_maps, core_ids=list(range(NCORES)),
                               trace=_trace)
    out = np.concatenate([res.results[i]["out"] for i in range(NCORES)], axis=0)
    kernel.last_exec_time_ns = res.exec_time_ns
    kernel.last_results = res
    return out.astype(np.float32)


# revision 62
# speedup vs baseline: 1.6537x; 1.0986x over previous
"""CTC prefix scorer on Trainium2 — Bass/Tile kernel, SPMD over 8 NeuronCores.

Math: the reference's 490-step lax.scan result is dead code (its output `r`
is only read at row start-1, which always comes from the LOGZERO prefix /
t=0 init), so the whole computation collapses to, per hypothesis h:

  log_psi[h, c] = log( sum_t w[t, h] * exp(x[b_h, t, c]) )

where w[t, h] = exp(rsum[t-1, h]) * [start <= t < xlen_b]  (normal labels)
          or  = exp(r_prev[t-1, 1, h]) * [...]             (c == last_ids[h])
with rsum = logaddexp(r_prev[:,0], r_prev[:,1]).  That is a (16 x T) @
(T x O) matmul per batch.  Frame masking folds into w (masked frames only
affect the BLANK/EOS output columns, which are overwritten anyway).  Final
output: scatter-select scored columns, EOS column = rsum[xlen-1], BLANK
column = LOGZERO, minus s_prev.

Sharding: core i <-> batch i (its 8 hypotheses).  x fully sharded on B.
"""

import numpy as np
from contextlib import ExitStack

import concourse.bass as bass
import concourse.tile as tile
from concourse import bacc, mybir
from concourse.bass import IndirectOffsetOnAxis
from concourse.bass_utils import run_bass_kernel_spmd
from concourse.tile_rust import add_dep_helper as _add_dep


def add_dep_helper(a, b, sync=True, reason=""):
    """a depends on b; unwrap BassInstruction -> mybir.Instruction."""
    _add_dep(getattr(a, "ins", a), getattr(b, "ins", b), sync=sync, reason=reason)

F32 = mybir.dt.float32
F32R = mybir.dt.float32r
I32 = mybir.dt.int32
ACT = mybir.ActivationFunctionType
ALU = mybir.AluOpType

B, T, O = 8, 500, 10000
NH = 8                       # hypotheses per batch == per core
NCORES = 8
LOGZERO = -1e10
BLANK, EOS = 0, 2
SNUM = 200

NT = 512                     # N-tile width (one PSUM bank of f32)
FLUSH = 5                    # N-tiles per deferred-Ln epilogue flush
N_TILES = [(c0, min(NT, O - c0)) for c0 in range(0, O, NT)]
K_CHUNKS = [(t0, min(128, T - t0)) for t0 in range(0, T, 128)]  # K over t<=499
NSCAT = (NH * SNUM + 127) // 128          # 13 indirect-scatter calls


def build_nc(start: int) -> bass.Bass:
    import os
    PHASE = os.environ.get("KDEBUG_PHASE", "full")
    nc = bacc.Bacc(None)
    x_d = nc.declare_dram_parameter("x", [T, O], F32, isOutput=False)
    rp_d = nc.declare_dram_parameter("rprev", [T, 2 * NH], F32, isOutput=False)
    sp_d = nc.declare_dram_parameter("sprev", [NH, O], F32, isOutput=False)
    li_d = nc.declare_dram_parameter("lastids", [NH, 1], I32, isOutput=False)
    mask_d = nc.declare_dram_parameter("smask", [NH, O], I32, isOutput=False)
    xl_d = nc.declare_dram_parameter("xlen", [128, 1], I32, isOutput=False)
    out_d = nc.declare_dram_parameter("out", [NH, O], F32, isOutput=True)

    with ExitStack() as ctx:
        tc = ctx.enter_context(tile.TileContext(nc))
        persist = ctx.enter_context(tc.tile_pool(name="persist", bufs=1))
        xpool = ctx.enter_context(tc.tile_pool(name="xp", bufs=8))
        psum = ctx.enter_context(tc.tile_pool(name="ps", bufs=4, space="PSUM"))
        psum_eos = ctx.enter_context(tc.tile_pool(name="pse", bufs=1, space="PSUM"))
        lap = ctx.enter_context(tc.tile_pool(name="lap", bufs=FLUSH))
        epi = ctx.enter_context(tc.tile_pool(name="epi", bufs=6))

        # ---------------- xlen broadcast ------------------------------------
        xlb = persist.tile([128, 1], I32, tag="xlb")
        nc.sync.dma_start(out=xlb[:], in_=xl_d[:, :])
        xlb_f = persist.tile([128, 1], F32, tag="xlbf")
        nc.vector.tensor_copy(out=xlb_f[:], in_=xlb[:])

        # ---------------- lhsT weights + eos --------------------------------
        # lhsT row t (global) <- r_prev[t-1]; chunk k covers t in [128k,128k+128)
        # eos[h] = rsum[xlen-1,h] = log(sum_t onehot[t]*sumexp[t,h]) computed
        # with an fp32r matmul (onehot broadcast to 256 cols to satisfy the
        # fp32r moving-dim restriction); row t holds rsum[t-1] so onehot is
        # at t == xlen.
        eos_acc = psum_eos.tile([NH, 256], F32)
        lhsTs = []
        for k, (t0, _) in enumerate(K_CHUNKS):
            a, b = max(t0, 1), min(t0 + 128, T + 1)
            pa, pb = a - t0, b - t0
            # full-128-partition ops only (SBUF compute APs must start at
            # partition 0): unloaded rows hold exp(0)=1 etc., neutralized by
            # the wm/oh masks below (always 0 there).
            e_t = persist.tile([128, 2 * NH], F32, tag=f"e{k}")
            nc.gpsimd.memset(e_t[:], 0.0)
            nc.sync.dma_start(out=e_t[pa:pb, :], in_=rp_d[a - 1:b - 1, :])
            nc.scalar.activation(e_t[:], e_t[:], ACT.Exp)
            sum_t = persist.tile([128, NH], F32, tag=f"sum{k}")
            nc.vector.tensor_tensor(out=sum_t[:], in0=e_t[:, 0:NH],
                                    in1=e_t[:, NH:2 * NH], op=ALU.add)

            io_t = persist.tile([128, 1], I32, tag=f"io{k}")
            nc.gpsimd.iota(io_t[:], pattern=[[0, 1]], base=t0, channel_multiplier=1)
            io_f = persist.tile([128, 1], F32, tag=f"iof{k}")
            nc.vector.tensor_copy(out=io_f[:], in_=io_t[:])
            ge_t = persist.tile([128, 1], F32, tag=f"ge{k}")
            nc.vector.tensor_scalar(out=ge_t[:], in0=io_f[:], scalar1=float(start),
                                    scalar2=None, op0=ALU.is_ge)
            lt_t = persist.tile([128, 1], F32, tag=f"lt{k}")
            nc.vector.tensor_scalar(out=lt_t[:], in0=io_f[:], scalar1=xlb_f[:, :1],
                                    scalar2=None, op0=ALU.is_lt)
            wm_t = persist.tile([128, 1], F32, tag=f"wm{k}")
            nc.vector.tensor_tensor(out=wm_t[:], in0=ge_t[:], in1=lt_t[:],
                                    op=ALU.mult)

            # eos matmul operands: onehot at t == xlen, broadcast to 256 cols
            oh_t = persist.tile([128, 1], F32, tag=f"oh{k}")
            nc.vector.tensor_scalar(out=oh_t[:], in0=io_f[:],
                                    scalar1=xlb_f[:, :1], scalar2=None,
                                    op0=ALU.is_equal)
            ohb_t = persist.tile([128, 256], F32R, tag=f"ohb{k}")
            nc.vector.tensor_scalar(out=ohb_t[:],
                                    in0=oh_t[:, :1].to_broadcast([128, 256]),
                                    scalar1=1.0, scalar2=None, op0=ALU.mult)
            sum_r = persist.tile([128, NH], F32R, tag=f"sumr{k}")
            nc.vector.tensor_copy(out=sum_r[:], in_=sum_t[:])
            nc.tensor.matmul(out=eos_acc[:], lhsT=sum_r[:], rhs=ohb_t[:],
                             start=(k == 0), stop=(k == len(K_CHUNKS) - 1))

            # w1 half lives at col 32 so the matmul output lands at PSUM
            # partition 32 (hardware requires partition starts in {0,32,64,96})
            # cols 8:32 are padding (psum partitions 8..31 unread); fill with
            # wm so their psum sums stay positive and Ln of the full tile is
            # finite (keeps the simulator's finiteness checks enabled).
            w_t = persist.tile([128, 32 + NH], F32R, tag=f"w{k}")
            nc.vector.tensor_scalar(out=w_t[:, NH:32],
                                    in0=wm_t[:, :1].to_broadcast([128, 32 - NH]),
                                    scalar1=1.0, scalar2=None, op0=ALU.mult)
            nc.vector.tensor_scalar(out=w_t[:, 0:NH], in0=sum_t[:], scalar1=wm_t[:, :1],
                                    scalar2=None, op0=ALU.mult)
            nc.vector.tensor_scalar(out=w_t[:, 32:32 + NH], in0=e_t[:, NH:2 * NH],
                                    scalar1=wm_t[:, :1], scalar2=None, op0=ALU.mult)
            lhsTs.append(w_t)

        # ---------------- shared epilogue constants -------------------------
        iotac_i = persist.tile([NH, NT], I32, tag="iotaci")
        nc.gpsimd.iota(iotac_i[:], pattern=[[1, NT]], base=0, channel_multiplier=0)
        iotac = persist.tile([NH, NT], F32, tag="iotac")
        nc.vector.tensor_copy(out=iotac[:], in_=iotac_i[:])
        li_t = persist.tile([NH, 1], I32, tag="li")
        nc.sync.dma_start(out=li_t[:], in_=li_d[:, :])
        li_f = persist.tile([NH, 1], F32, tag="lif")
        nc.vector.tensor_copy(out=li_f[:], in_=li_t[:])

        # ---------------- main loop: deferred-Ln flushes ---------------------
        # Per tile: DMA + Exp + matmul, then a cheap DVE copy PSUM->SBUF that
        # frees the psum bank (so the scheduler never hoists Ln to relieve
        # PSUM pressure).  Every FLUSH tiles, run the batched Lns (one ACT
        # table swap per batch instead of two per tile) and the epilogues.
        eos_sb = persist.tile([NH, 1], F32, tag="eos")
        eos_done = False
        pend = []

        def flush_pend():
            nonlocal eos_done
            for j, c0, N, la in pend:
                nc.scalar.activation(la[:, :N], la[:, :N], ACT.Ln)
            if not eos_done:
                nc.scalar.activation(eos_sb[:], eos_acc[0:NH, 0:1], ACT.Ln)
                eos_done = True
            for j, c0, N, la in pend:
                lastc0 = epi.tile([NH, 1], F32, tag="lastc0")
                nc.vector.tensor_scalar(out=lastc0[:], in0=li_f[:],
                                        scalar1=float(c0), scalar2=None,
                                        op0=ALU.subtract)
                # hit mask written at base partition 32 so mask and data of
                # copy_predicated share a base partition (hardware requires
                # equal SBUF input bases for 2-input DVE ops)
                hit = epi.tile([32 + NH, NT], I32, tag="hit")
                nc.vector.tensor_scalar(out=hit[32:32 + NH, :N], in0=iotac[:, :N],
                                        scalar1=lastc0[:, :1], scalar2=None,
                                        op0=ALU.is_equal)
                # merge the hit column into la0 in place
                nc.vector.copy_predicated(out=la[0:NH, :N],
                                          mask=hit[32:32 + NH, :N],
                                          data=la[32:32 + NH, :N])

                sm = epi.tile([NH, NT], I32, tag="sm")
                nc.sync.dma_start(out=sm[:, :N], in_=mask_d[:, c0:c0 + N])
                spv = epi.tile([NH, NT], F32, tag="spv")
                nc.sync.dma_start(out=spv[:, :N], in_=sp_d[:, c0:c0 + N])

                val2 = epi.tile([NH, NT], F32, tag="val2")
                nc.vector.tensor_tensor(out=val2[:, :N], in0=la[0:NH, :N],
                                        in1=spv[:, :N], op=ALU.subtract)
                fin = epi.tile([NH, NT], F32, tag="fin")
                nc.vector.tensor_scalar(out=fin[:, :N], in0=spv[:, :N],
                                        scalar1=-1.0, scalar2=LOGZERO,
                                        op0=ALU.mult, op1=ALU.add)
                nc.vector.copy_predicated(out=fin[:, :N], mask=sm[:, :N],
                                          data=val2[:, :N])
                if j == 0:
                    # EOS col: eos - s_prev (BLANK col already LOGZERO - s_prev
                    # since smask[BLANK]=0 is forced host-side)
                    nc.vector.tensor_tensor(out=fin[:, EOS:EOS + 1],
                                            in0=eos_sb[:],
                                            in1=spv[:, EOS:EOS + 1],
                                            op=ALU.subtract)
                nc.sync.dma_start(out=out_d[:, c0:c0 + N], in_=fin[:, :N])
            pend.clear()

        for j, (c0, N) in enumerate(N_TILES):
            acc = psum.tile([32 + NH, NT], F32, tag="acc")
            for k, (t0, K) in enumerate(K_CHUNKS):
                xraw = xpool.tile([128, NT], F32, tag="xraw")
                nc.sync.dma_start(out=xraw[:K, :N],
                                  in_=x_d[t0:t0 + K, c0:c0 + N])
                xt = xpool.tile([128, NT], F32R, tag="xt")
                nc.scalar.activation(xt[:K, :N], xraw[:K, :N], ACT.Exp)
                nc.tensor.matmul(out=acc[:, :N], lhsT=lhsTs[k][:K, :],
                                 rhs=xt[:K, :N],
                                 start=(k == 0), stop=(k == len(K_CHUNKS) - 1))
            la = lap.tile([32 + NH, NT], F32, tag="la")
            nc.vector.tensor_copy(out=la[:, :N], in_=acc[:, :N])
            pend.append((j, c0, N, la))
            if len(pend) == FLUSH or j == len(N_TILES) - 1:
                flush_pend()

    nc.compile()
    return nc


def make_in_maps(x, r_prev, s_prev, xlens, last_ids, scoring_ids):
    """Per-core input maps: core i owns batch i / hypotheses [8i, 8i+8)."""
    in_maps = []
    for i in range(NCORES):
        hs = slice(i * NH, (i + 1) * NH)
        sids = np.ascontiguousarray(scoring_ids[hs]).astype(np.int64)  # (8,200)
        smask = np.zeros((NH, O), np.int32)
        np.put_along_axis(smask, sids, 1, axis=1)
        smask[:, BLANK] = 0  # BLANK output column is always LOGZERO - s_prev
        in_maps.append({
            "x": np.ascontiguousarray(x[i]).astype(np.float32),
            "rprev": np.ascontiguousarray(r_prev[:, :, hs]).reshape(T, 2 * NH).astype(np.float32),
            "sprev": np.ascontiguousarray(s_prev[hs]).astype(np.float32),
            "lastids": np.ascontiguousarray(last_ids[hs]).astype(np.int32)[:, None],
            "smask": smask,
            "xlen": np.full((128, 1), int(xlens[i]), np.int32),
        })
    return in_maps


_NC_CACHE: dict[int, bass.Bass] = {}


def kernel(x, r_prev, s_prev, xlens, last_ids, scoring_ids, output_length,
           _trace=False):
    x = np.asarray(x)
    r_prev = np.asarray(r_prev)
    s_prev = np.asarray(s_prev)
    xlens = np.asarray(xlens)
    last_ids = np.asarray(last_ids)
    scoring_ids = np.asarray(scoring_ids)
    start = max(int(output_length), 1)
    # output_length == 0 adds an extra x_[0,0] term; inputs here always have
    # output_length >= 1, which this kernel implements.
    assert int(output_length) >= 1, "output_length==0 path not implemented"

    if start not in _NC_CACHE:
        _NC_CACHE[start] = build_nc(start)
    nc = _NC_CACHE[start]

    in_maps = make_in_maps(x, r_prev, s_prev, xlens, last_ids, scoring_ids)
    res = run_bass_kernel_spmd(nc, in_maps, core_ids=list(range(NCORES)),
                               trace=_trace)
    out = np.concatenate([res.results[i]["out"] for i in range(NCORES)], axis=0)
    kernel.last_exec_time_ns = res.exec_time_ns
    kernel.last_results = res
    return out.astype(np.float32)


# revision 64
# speedup vs baseline: 1.7465x; 1.0561x over previous
"""CTC prefix scorer on Trainium2 — Bass/Tile kernel, SPMD over 8 NeuronCores.

Math: the reference's 490-step lax.scan result is dead code (its output `r`
is only read at row start-1, which always comes from the LOGZERO prefix /
t=0 init), so the whole computation collapses to, per hypothesis h:

  log_psi[h, c] = log( sum_t w[t, h] * exp(x[b_h, t, c]) )

where w[t, h] = exp(rsum[t-1, h]) * [start <= t < xlen_b]  (normal labels)
          or  = exp(r_prev[t-1, 1, h]) * [...]             (c == last_ids[h])
with rsum = logaddexp(r_prev[:,0], r_prev[:,1]).  That is a (16 x T) @
(T x O) matmul per batch.  Frame masking folds into w (masked frames only
affect the BLANK/EOS output columns, which are overwritten anyway).  Final
output: scatter-select scored columns, EOS column = rsum[xlen-1], BLANK
column = LOGZERO, minus s_prev.

Sharding: core i <-> batch i (its 8 hypotheses).  x fully sharded on B.
"""

import numpy as np
from contextlib import ExitStack

import concourse.bass as bass
import concourse.tile as tile
from concourse import bacc, mybir
from concourse.bass import IndirectOffsetOnAxis
from concourse.bass_utils import run_bass_kernel_spmd
from concourse.tile_rust import add_dep_helper as _add_dep


def add_dep_helper(a, b, sync=True, reason=""):
    """a depends on b; unwrap BassInstruction -> mybir.Instruction."""
    _add_dep(getattr(a, "ins", a), getattr(b, "ins", b), sync=sync, reason=reason)

F32 = mybir.dt.float32
F32R = mybir.dt.float32r
I32 = mybir.dt.int32
ACT = mybir.ActivationFunctionType
ALU = mybir.AluOpType

B, T, O = 8, 500, 10000
NH = 8                       # hypotheses per batch == per core
NCORES = 8
LOGZERO = -1e10
BLANK, EOS = 0, 2
SNUM = 200

NT = 512                     # N-tile width (one PSUM bank of f32)
FLUSH = 5                    # N-tiles per deferred-Ln epilogue flush
N_TILES = [(c0, min(NT, O - c0)) for c0 in range(0, O, NT)]
K_CHUNKS = [(t0, min(128, T - t0)) for t0 in range(0, T, 128)]  # K over t<=499
NSCAT = (NH * SNUM + 127) // 128          # 13 indirect-scatter calls


def build_nc(start: int) -> bass.Bass:
    import os
    PHASE = os.environ.get("KDEBUG_PHASE", "full")
    nc = bacc.Bacc(None)
    x_d = nc.declare_dram_parameter("x", [T, O], F32, isOutput=False)
    rp_d = nc.declare_dram_parameter("rprev", [T, 2 * NH], F32, isOutput=False)
    sp_d = nc.declare_dram_parameter("sprev", [NH, O], F32, isOutput=False)
    li_d = nc.declare_dram_parameter("lastids", [NH, 1], I32, isOutput=False)
    mask_d = nc.declare_dram_parameter("smask", [NH, O], I32, isOutput=False)
    xl_d = nc.declare_dram_parameter("xlen", [128, 1], I32, isOutput=False)
    out_d = nc.declare_dram_parameter("out", [NH, O], F32, isOutput=True)

    with ExitStack() as ctx:
        tc = ctx.enter_context(tile.TileContext(nc))
        persist = ctx.enter_context(tc.tile_pool(name="persist", bufs=1))
        xpool = ctx.enter_context(tc.tile_pool(name="xp", bufs=8))
        psum = ctx.enter_context(tc.tile_pool(name="ps", bufs=4, space="PSUM"))
        psum_eos = ctx.enter_context(tc.tile_pool(name="pse", bufs=1, space="PSUM"))
        lap = ctx.enter_context(tc.tile_pool(name="lap", bufs=FLUSH))
        epi = ctx.enter_context(tc.tile_pool(name="epi", bufs=6))

        # ---------------- xlen broadcast ------------------------------------
        xlb = persist.tile([128, 1], I32, tag="xlb")
        nc.sync.dma_start(out=xlb[:], in_=xl_d[:, :])
        xlb_f = persist.tile([128, 1], F32, tag="xlbf")
        nc.vector.tensor_copy(out=xlb_f[:], in_=xlb[:])

        # ---------------- lhsT weights + eos --------------------------------
        # lhsT row t (global) <- r_prev[t-1]; chunk k covers t in [128k,128k+128)
        # eos[h] = rsum[xlen-1,h] = log(sum_t onehot[t]*sumexp[t,h]) computed
        # with an fp32r matmul (onehot broadcast to 256 cols to satisfy the
        # fp32r moving-dim restriction); row t holds rsum[t-1] so onehot is
        # at t == xlen.
        eos_acc = psum_eos.tile([NH, 256], F32)
        lhsTs = []
        for k, (t0, _) in enumerate(K_CHUNKS):
            a, b = max(t0, 1), min(t0 + 128, T + 1)
            pa, pb = a - t0, b - t0
            # full-128-partition ops only (SBUF compute APs must start at
            # partition 0): unloaded rows hold exp(0)=1 etc., neutralized by
            # the wm/oh masks below (always 0 there).
            e_t = persist.tile([128, 2 * NH], F32, tag=f"e{k}")
            nc.gpsimd.memset(e_t[:], 0.0)
            nc.sync.dma_start(out=e_t[pa:pb, :], in_=rp_d[a - 1:b - 1, :])
            nc.scalar.activation(e_t[:], e_t[:], ACT.Exp)
            sum_t = persist.tile([128, NH], F32, tag=f"sum{k}")
            nc.vector.tensor_tensor(out=sum_t[:], in0=e_t[:, 0:NH],
                                    in1=e_t[:, NH:2 * NH], op=ALU.add)

            io_t = persist.tile([128, 1], I32, tag=f"io{k}")
            nc.gpsimd.iota(io_t[:], pattern=[[0, 1]], base=t0, channel_multiplier=1)
            io_f = persist.tile([128, 1], F32, tag=f"iof{k}")
            nc.vector.tensor_copy(out=io_f[:], in_=io_t[:])
            ge_t = persist.tile([128, 1], F32, tag=f"ge{k}")
            nc.vector.tensor_scalar(out=ge_t[:], in0=io_f[:], scalar1=float(start),
                                    scalar2=None, op0=ALU.is_ge)
            lt_t = persist.tile([128, 1], F32, tag=f"lt{k}")
            nc.vector.tensor_scalar(out=lt_t[:], in0=io_f[:], scalar1=xlb_f[:, :1],
                                    scalar2=None, op0=ALU.is_lt)
            wm_t = persist.tile([128, 1], F32, tag=f"wm{k}")
            nc.vector.tensor_tensor(out=wm_t[:], in0=ge_t[:], in1=lt_t[:],
                                    op=ALU.mult)

            # eos matmul operands: onehot at t == xlen, broadcast to 256 cols
            oh_t = persist.tile([128, 1], F32, tag=f"oh{k}")
            nc.vector.tensor_scalar(out=oh_t[:], in0=io_f[:],
                                    scalar1=xlb_f[:, :1], scalar2=None,
                                    op0=ALU.is_equal)
            ohb_t = persist.tile([128, 256], F32R, tag=f"ohb{k}")
            nc.vector.tensor_scalar(out=ohb_t[:],
                                    in0=oh_t[:, :1].to_broadcast([128, 256]),
                                    scalar1=1.0, scalar2=None, op0=ALU.mult)
            sum_r = persist.tile([128, NH], F32R, tag=f"sumr{k}")
            nc.vector.tensor_copy(out=sum_r[:], in_=sum_t[:])
            nc.tensor.matmul(out=eos_acc[:], lhsT=sum_r[:], rhs=ohb_t[:],
                             start=(k == 0), stop=(k == len(K_CHUNKS) - 1))

            # w1 half lives at col 32 so the matmul output lands at PSUM
            # partition 32 (hardware requires partition starts in {0,32,64,96})
            # cols 8:32 are padding (psum partitions 8..31 unread); fill with
            # wm so their psum sums stay positive and Ln of the full tile is
            # finite (keeps the simulator's finiteness checks enabled).
            w_t = persist.tile([128, 32 + NH], F32R, tag=f"w{k}")
            nc.vector.tensor_scalar(out=w_t[:, NH:32],
                                    in0=wm_t[:, :1].to_broadcast([128, 32 - NH]),
                                    scalar1=1.0, scalar2=None, op0=ALU.mult)
            nc.vector.tensor_scalar(out=w_t[:, 0:NH], in0=sum_t[:], scalar1=wm_t[:, :1],
                                    scalar2=None, op0=ALU.mult)
            nc.vector.tensor_scalar(out=w_t[:, 32:32 + NH], in0=e_t[:, NH:2 * NH],
                                    scalar1=wm_t[:, :1], scalar2=None, op0=ALU.mult)
            lhsTs.append(w_t)

        # ---------------- shared epilogue constants -------------------------
        iotac_i = persist.tile([NH, NT], I32, tag="iotaci")
        nc.gpsimd.iota(iotac_i[:], pattern=[[1, NT]], base=0, channel_multiplier=0)
        iotac = persist.tile([NH, NT], F32, tag="iotac")
        nc.vector.tensor_copy(out=iotac[:], in_=iotac_i[:])
        li_t = persist.tile([NH, 1], I32, tag="li")
        nc.sync.dma_start(out=li_t[:], in_=li_d[:, :])
        li_f = persist.tile([NH, 1], F32, tag="lif")
        nc.vector.tensor_copy(out=li_f[:], in_=li_t[:])

        # ---------------- main loop: deferred-Ln flushes ---------------------
        # Per tile: DMA + Exp + matmul, then a cheap DVE copy PSUM->SBUF that
        # frees the psum bank (so the scheduler never hoists Ln to relieve
        # PSUM pressure).  Every FLUSH tiles, run the batched Lns (one ACT
        # table swap per batch instead of two per tile) and the epilogues.
        eos_sb = persist.tile([NH, 1], F32, tag="eos")
        eos_done = False
        pend = []
        window_exps = []
        prev_last_ln = [None]

        def flush_pend():
            nonlocal eos_done
            # Pin the ACT stream order: Lns after this window's last Exp.
            # Same-engine deps are pure ordering (no semaphores) but stop the
            # scheduler hoisting Lns into Exp runs, which costs a 1.3us
            # activation-table reload each way.
            last_exp = window_exps[-1] if window_exps else None
            lns = []
            for j, c0, N, la in pend:
                li2 = nc.scalar.activation(la[:, :N], la[:, :N], ACT.Ln)
                if last_exp is not None:
                    add_dep_helper(li2, last_exp, sync=True,
                                   reason="ACT table batching")
                lns.append(li2)
            if not eos_done:
                li2 = nc.scalar.activation(eos_sb[:], eos_acc[0:NH, 0:1], ACT.Ln)
                if last_exp is not None:
                    add_dep_helper(li2, last_exp, sync=True,
                                   reason="ACT table batching")
                lns.append(li2)
                eos_done = True
            prev_last_ln[0] = lns[-1] if lns else None
            window_exps.clear()
            for j, c0, N, la in pend:
                lastc0 = epi.tile([NH, 1], F32, tag="lastc0")
                nc.vector.tensor_scalar(out=lastc0[:], in0=li_f[:],
                                        scalar1=float(c0), scalar2=None,
                                        op0=ALU.subtract)
                # hit mask written at base partition 32 so mask and data of
                # copy_predicated share a base partition (hardware requires
                # equal SBUF input bases for 2-input DVE ops)
                hit = epi.tile([32 + NH, NT], I32, tag="hit")
                nc.vector.tensor_scalar(out=hit[32:32 + NH, :N], in0=iotac[:, :N],
                                        scalar1=lastc0[:, :1], scalar2=None,
                                        op0=ALU.is_equal)
                # merge the hit column into la0 in place
                nc.vector.copy_predicated(out=la[0:NH, :N],
                                          mask=hit[32:32 + NH, :N],
                                          data=la[32:32 + NH, :N])

                sm = epi.tile([NH, NT], I32, tag="sm")
                nc.sync.dma_start(out=sm[:, :N], in_=mask_d[:, c0:c0 + N])
                spv = epi.tile([NH, NT], F32, tag="spv")
                nc.sync.dma_start(out=spv[:, :N], in_=sp_d[:, c0:c0 + N])

                val2 = epi.tile([NH, NT], F32, tag="val2")
                nc.vector.tensor_tensor(out=val2[:, :N], in0=la[0:NH, :N],
                                        in1=spv[:, :N], op=ALU.subtract)
                fin = epi.tile([NH, NT], F32, tag="fin")
                nc.vector.tensor_scalar(out=fin[:, :N], in0=spv[:, :N],
                                        scalar1=-1.0, scalar2=LOGZERO,
                                        op0=ALU.mult, op1=ALU.add)
                nc.vector.copy_predicated(out=fin[:, :N], mask=sm[:, :N],
                                          data=val2[:, :N])
                if j == 0:
                    # EOS col: eos - s_prev (BLANK col already LOGZERO - s_prev
                    # since smask[BLANK]=0 is forced host-side)
                    nc.vector.tensor_tensor(out=fin[:, EOS:EOS + 1],
                                            in0=eos_sb[:],
                                            in1=spv[:, EOS:EOS + 1],
                                            op=ALU.subtract)
                nc.sync.dma_start(out=out_d[:, c0:c0 + N], in_=fin[:, :N])
            pend.clear()

        for j, (c0, N) in enumerate(N_TILES):
            acc = psum.tile([32 + NH, NT], F32, tag="acc")
            for k, (t0, K) in enumerate(K_CHUNKS):
                xraw = xpool.tile([128, NT], F32, tag="xraw")
                nc.sync.dma_start(out=xraw[:K, :N],
                                  in_=x_d[t0:t0 + K, c0:c0 + N])
                xt = xpool.tile([128, NT], F32R, tag="xt")
                ei = nc.scalar.activation(xt[:K, :N], xraw[:K, :N], ACT.Exp)
                if prev_last_ln[0] is not None:
                    add_dep_helper(ei, prev_last_ln[0], sync=True,
                                   reason="ACT table batching")
                window_exps.append(ei)
                nc.tensor.matmul(out=acc[:, :N], lhsT=lhsTs[k][:K, :],
                                 rhs=xt[:K, :N],
                                 start=(k == 0), stop=(k == len(K_CHUNKS) - 1))
            la = lap.tile([32 + NH, NT], F32, tag="la")
            nc.vector.tensor_copy(out=la[:, :N], in_=acc[:, :N])
            pend.append((j, c0, N, la))
            if len(pend) == FLUSH or j == len(N_TILES) - 1:
                flush_pend()

    nc.compile()
    return nc


def make_in_maps(x, r_prev, s_prev, xlens, last_ids, scoring_ids):
    """Per-core input maps: core i owns batch i / hypotheses [8i, 8i+8)."""
    in_maps = []
    for i in range(NCORES):
        hs = slice(i * NH, (i + 1) * NH)
        sids = np.ascontiguousarray(scoring_ids[hs]).astype(np.int64)  # (8,200)
        smask = np.zeros((NH, O), np.int32)
        np.put_along_axis(smask, sids, 1, axis=1)
        smask[:, BLANK] = 0  # BLANK output column is always LOGZERO - s_prev
        in_maps.append({
            "x": np.ascontiguousarray(x[i]).astype(np.float32),
            "rprev": np.ascontiguousarray(r_prev[:, :, hs]).reshape(T, 2 * NH).astype(np.float32),
            "sprev": np.ascontiguousarray(s_prev[hs]).astype(np.float32),
            "lastids": np.ascontiguousarray(last_ids[hs]).astype(np.int32)[:, None],
            "smask": smask,
            "xlen": np.full((128, 1), int(xlens[i]), np.int32),
        })
    return in_maps


_NC_CACHE: dict[int, bass.Bass] = {}


def kernel(x, r_prev, s_prev, xlens, last_ids, scoring_ids, output_length,
           _trace=False):
    x = np.asarray(x)
    r_prev = np.asarray(r_prev)
    s_prev = np.asarray(s_prev)
    xlens = np.asarray(xlens)
    last_ids = np.asarray(last_ids)
    scoring_ids = np.asarray(scoring_ids)
    start = max(int(output_length), 1)
    # output_length == 0 adds an extra x_[0,0] term; inputs here always have
    # output_length >= 1, which this kernel implements.
    assert int(output_length) >= 1, "output_length==0 path not implemented"

    if start not in _NC_CACHE:
        _NC_CACHE[start] = build_nc(start)
    nc = _NC_CACHE[start]

    in_maps = make_in_maps(x, r_prev, s_prev, xlens, last_ids, scoring_ids)
    res = run_bass_kernel_spmd(nc, in_maps, core_ids=list(range(NCORES)),
                               trace=_trace)
    out = np.concatenate([res.results[i]["out"] for i in range(NCORES)], axis=0)
    kernel.last_exec_time_ns = res.exec_time_ns
    kernel.last_results = res
    return out.astype(np.float32)


# revision 70
# speedup vs baseline: 1.8436x; 1.0556x over previous
"""CTC prefix scorer on Trainium2 — Bass/Tile kernel, SPMD over 8 NeuronCores.

Math: the reference's 490-step lax.scan result is dead code (its output `r`
is only read at row start-1, which always comes from the LOGZERO prefix /
t=0 init), so the whole computation collapses to, per hypothesis h:

  log_psi[h, c] = log( sum_t w[t, h] * exp(x[b_h, t, c]) )

where w[t, h] = exp(rsum[t-1, h]) * [start <= t < xlen_b]  (normal labels)
          or  = exp(r_prev[t-1, 1, h]) * [...]             (c == last_ids[h])
with rsum = logaddexp(r_prev[:,0], r_prev[:,1]).  That is a (16 x T) @
(T x O) matmul per batch.  Frame masking folds into w (masked frames only
affect the BLANK/EOS output columns, which are overwritten anyway).  Final
output: scatter-select scored columns, EOS column = rsum[xlen-1], BLANK
column = LOGZERO, minus s_prev.

Sharding: core i <-> batch i (its 8 hypotheses).  x fully sharded on B.
"""

import numpy as np
from contextlib import ExitStack

import concourse.bass as bass
import concourse.tile as tile
from concourse import bacc, mybir
from concourse.bass import IndirectOffsetOnAxis
from concourse.bass_utils import run_bass_kernel_spmd
from concourse.tile_rust import add_dep_helper as _add_dep


def add_dep_helper(a, b, sync=True, reason=""):
    """a depends on b; unwrap BassInstruction -> mybir.Instruction."""
    _add_dep(getattr(a, "ins", a), getattr(b, "ins", b), sync=sync, reason=reason)

F32 = mybir.dt.float32
F32R = mybir.dt.float32r
I32 = mybir.dt.int32
ACT = mybir.ActivationFunctionType
ALU = mybir.AluOpType

B, T, O = 8, 500, 10000
NH = 8                       # hypotheses per batch == per core
NCORES = 8
LOGZERO = -1e10
BLANK, EOS = 0, 2
SNUM = 200

NT = 512                     # N-tile width (one PSUM bank of f32)
WLOAD = 2048                 # x load-group width (8KB DMA rows)
FLUSH = 5                    # la tiles live per group (4 + 1 slack)
N_TILES = [(c0, min(NT, O - c0)) for c0 in range(0, O, NT)]
K_CHUNKS = [(t0, min(128, T - t0)) for t0 in range(0, T, 128)]  # K over t<=499
NSCAT = (NH * SNUM + 127) // 128          # 13 indirect-scatter calls


def build_nc(start: int) -> bass.Bass:
    import os
    PHASE = os.environ.get("KDEBUG_PHASE", "full")
    nc = bacc.Bacc(None)
    x_d = nc.declare_dram_parameter("x", [T, O], F32, isOutput=False)
    rp_d = nc.declare_dram_parameter("rprev", [T, 2 * NH], F32, isOutput=False)
    sp_d = nc.declare_dram_parameter("sprev", [NH, O], F32, isOutput=False)
    li_d = nc.declare_dram_parameter("lastids", [NH, 1], I32, isOutput=False)
    mask_d = nc.declare_dram_parameter("smask", [NH, O], I32, isOutput=False)
    xl_d = nc.declare_dram_parameter("xlen", [128, 1], I32, isOutput=False)
    out_d = nc.declare_dram_parameter("out", [NH, O], F32, isOutput=True)

    with ExitStack() as ctx:
        tc = ctx.enter_context(tile.TileContext(nc))
        persist = ctx.enter_context(tc.tile_pool(name="persist", bufs=1))
        xpool = ctx.enter_context(tc.tile_pool(name="xp", bufs=5))
        psum = ctx.enter_context(tc.tile_pool(name="ps", bufs=4, space="PSUM"))
        psum_eos = ctx.enter_context(tc.tile_pool(name="pse", bufs=1, space="PSUM"))
        lap = ctx.enter_context(tc.tile_pool(name="lap", bufs=FLUSH))
        epi = ctx.enter_context(tc.tile_pool(name="epi", bufs=2))
        epis = ctx.enter_context(tc.tile_pool(name="epis", bufs=6))

        # ---------------- xlen broadcast ------------------------------------
        xlb = persist.tile([128, 1], I32, tag="xlb")
        nc.sync.dma_start(out=xlb[:], in_=xl_d[:, :])
        xlb_f = persist.tile([128, 1], F32, tag="xlbf")
        nc.vector.tensor_copy(out=xlb_f[:], in_=xlb[:])

        # ---------------- lhsT weights + eos --------------------------------
        # lhsT row t (global) <- r_prev[t-1]; chunk k covers t in [128k,128k+128)
        # eos[h] = rsum[xlen-1,h] = log(sum_t onehot[t]*sumexp[t,h]) computed
        # with an fp32r matmul (onehot broadcast to 256 cols to satisfy the
        # fp32r moving-dim restriction); row t holds rsum[t-1] so onehot is
        # at t == xlen.
        eos_acc = psum_eos.tile([NH, 256], F32)
        lhsTs = []
        for k, (t0, _) in enumerate(K_CHUNKS):
            a, b = max(t0, 1), min(t0 + 128, T + 1)
            pa, pb = a - t0, b - t0
            # full-128-partition ops only (SBUF compute APs must start at
            # partition 0): unloaded rows hold exp(0)=1 etc., neutralized by
            # the wm/oh masks below (always 0 there).
            e_t = persist.tile([128, 2 * NH], F32, tag=f"e{k}")
            nc.gpsimd.memset(e_t[:], 0.0)
            nc.sync.dma_start(out=e_t[pa:pb, :], in_=rp_d[a - 1:b - 1, :])
            nc.scalar.activation(e_t[:], e_t[:], ACT.Exp)
            sum_t = persist.tile([128, NH], F32, tag=f"sum{k}")
            nc.vector.tensor_tensor(out=sum_t[:], in0=e_t[:, 0:NH],
                                    in1=e_t[:, NH:2 * NH], op=ALU.add)

            io_t = persist.tile([128, 1], I32, tag=f"io{k}")
            nc.gpsimd.iota(io_t[:], pattern=[[0, 1]], base=t0, channel_multiplier=1)
            io_f = persist.tile([128, 1], F32, tag=f"iof{k}")
            nc.vector.tensor_copy(out=io_f[:], in_=io_t[:])
            ge_t = persist.tile([128, 1], F32, tag=f"ge{k}")
            nc.vector.tensor_scalar(out=ge_t[:], in0=io_f[:], scalar1=float(start),
                                    scalar2=None, op0=ALU.is_ge)
            lt_t = persist.tile([128, 1], F32, tag=f"lt{k}")
            nc.vector.tensor_scalar(out=lt_t[:], in0=io_f[:], scalar1=xlb_f[:, :1],
                                    scalar2=None, op0=ALU.is_lt)
            wm_t = persist.tile([128, 1], F32, tag=f"wm{k}")
            nc.vector.tensor_tensor(out=wm_t[:], in0=ge_t[:], in1=lt_t[:],
                                    op=ALU.mult)

            # eos matmul operands: onehot at t == xlen, broadcast to 256 cols
            oh_t = persist.tile([128, 1], F32, tag=f"oh{k}")
            nc.vector.tensor_scalar(out=oh_t[:], in0=io_f[:],
                                    scalar1=xlb_f[:, :1], scalar2=None,
                                    op0=ALU.is_equal)
            ohb_t = persist.tile([128, 256], F32R, tag=f"ohb{k}")
            nc.vector.tensor_scalar(out=ohb_t[:],
                                    in0=oh_t[:, :1].to_broadcast([128, 256]),
                                    scalar1=1.0, scalar2=None, op0=ALU.mult)
            sum_r = persist.tile([128, NH], F32R, tag=f"sumr{k}")
            nc.vector.tensor_copy(out=sum_r[:], in_=sum_t[:])
            nc.tensor.matmul(out=eos_acc[:], lhsT=sum_r[:], rhs=ohb_t[:],
                             start=(k == 0), stop=(k == len(K_CHUNKS) - 1))

            # w1 half lives at col 32 so the matmul output lands at PSUM
            # partition 32 (hardware requires partition starts in {0,32,64,96})
            # cols 8:32 are padding (psum partitions 8..31 unread); fill with
            # wm so their psum sums stay positive and Ln of the full tile is
            # finite (keeps the simulator's finiteness checks enabled).
            w_t = persist.tile([128, 32 + NH], F32R, tag=f"w{k}")
            nc.vector.tensor_scalar(out=w_t[:, NH:32],
                                    in0=wm_t[:, :1].to_broadcast([128, 32 - NH]),
                                    scalar1=1.0, scalar2=None, op0=ALU.mult)
            nc.vector.tensor_scalar(out=w_t[:, 0:NH], in0=sum_t[:], scalar1=wm_t[:, :1],
                                    scalar2=None, op0=ALU.mult)
            nc.vector.tensor_scalar(out=w_t[:, 32:32 + NH], in0=e_t[:, NH:2 * NH],
                                    scalar1=wm_t[:, :1], scalar2=None, op0=ALU.mult)
            lhsTs.append(w_t)

        # ---------------- shared epilogue constants -------------------------
        iotac_i = persist.tile([NH, NT], I32, tag="iotaci")
        nc.gpsimd.iota(iotac_i[:], pattern=[[1, NT]], base=0, channel_multiplier=0)
        iotac = persist.tile([NH, NT], F32, tag="iotac")
        nc.vector.tensor_copy(out=iotac[:], in_=iotac_i[:])
        li_t = persist.tile([NH, 1], I32, tag="li")
        nc.sync.dma_start(out=li_t[:], in_=li_d[:, :])
        li_f = persist.tile([NH, 1], F32, tag="lif")
        nc.vector.tensor_copy(out=li_f[:], in_=li_t[:])

        # ---------------- main loop: deferred-Ln flushes ---------------------
        # Per tile: DMA + Exp + matmul, then a cheap DVE copy PSUM->SBUF that
        # frees the psum bank (so the scheduler never hoists Ln to relieve
        # PSUM pressure).  Every FLUSH tiles, run the batched Lns (one ACT
        # table swap per batch instead of two per tile) and the epilogues.
        # ---------------- main loop: wide load groups ------------------------
        # x is loaded in (128, 2048) chunks (8KB contiguous rows -> few, fat
        # DMA descriptors spread evenly over the HWDGE queues), exp'd once per
        # chunk, then consumed by 4 per-512-subtile matmul accumulations.
        # Lns are deferred per group (cheap DVE psum->SBUF copies free the
        # banks) and pinned after the group's Exps so the ACT activation
        # table swaps only twice per group.
        eos_sb = persist.tile([NH, 1], F32, tag="eos")
        eos_done = False
        prev_last_ln = None

        for g0 in range(0, O, WLOAD):
            W = min(WLOAD, O - g0)
            xts = []
            group_exps = []
            for k, (t0, K) in enumerate(K_CHUNKS):
                xraw = xpool.tile([128, WLOAD], F32, tag="xraw")
                nc.sync.dma_start(out=xraw[:K, :W],
                                  in_=x_d[t0:t0 + K, g0:g0 + W])
                xt = xpool.tile([128, WLOAD], F32R, tag="xt")
                ei = nc.scalar.activation(xt[:K, :W], xraw[:K, :W], ACT.Exp)
                if prev_last_ln is not None:
                    add_dep_helper(ei, prev_last_ln, sync=True,
                                   reason="ACT table batching")
                group_exps.append(ei)
                xts.append(xt)

            sm_w = epi.tile([NH, WLOAD], I32, tag="smw")
            nc.sync.dma_start(out=sm_w[:, :W], in_=mask_d[:, g0:g0 + W])
            spv_w = epi.tile([NH, WLOAD], F32, tag="spvw")
            nc.sync.dma_start(out=spv_w[:, :W], in_=sp_d[:, g0:g0 + W])
            fin_w = epi.tile([NH, WLOAD], F32, tag="finw")

            las = []
            for s0 in range(0, W, NT):
                N = min(NT, W - s0)
                acc = psum.tile([32 + NH, NT], F32, tag="acc")
                for k, (t0, K) in enumerate(K_CHUNKS):
                    nc.tensor.matmul(out=acc[:, :N], lhsT=lhsTs[k][:K, :],
                                     rhs=xts[k][:K, s0:s0 + N],
                                     start=(k == 0),
                                     stop=(k == len(K_CHUNKS) - 1))
                la = lap.tile([32 + NH, NT], F32, tag="la")
                nc.vector.tensor_copy(out=la[:, :N], in_=acc[:, :N])
                las.append((s0, N, la))

            # batched Lns, pinned after this group's Exps (same-engine deps =
            # pure ordering; stops 1.3us activation-table thrash)
            last_exp = group_exps[-1]
            lns = []
            for s0, N, la in las:
                li2 = nc.scalar.activation(la[:, :N], la[:, :N], ACT.Ln)
                add_dep_helper(li2, last_exp, sync=True,
                               reason="ACT table batching")
                lns.append(li2)
            if not eos_done:
                li2 = nc.scalar.activation(eos_sb[:], eos_acc[0:NH, 0:1], ACT.Ln)
                add_dep_helper(li2, last_exp, sync=True,
                               reason="ACT table batching")
                lns.append(li2)
                eos_done = True
            prev_last_ln = lns[-1]

            for s0, N, la in las:
                c0 = g0 + s0
                # hit[h,c] = (c == last_ids[h]); written at base partition 32
                # so copy_predicated's mask and data share a base partition
                hit = epis.tile([32 + NH, NT], I32, tag="hit")
                nc.vector.tensor_scalar(out=hit[32:32 + NH, :N],
                                        in0=iotac[:, :N],
                                        scalar1=li_f[:, :1],
                                        scalar2=float(-c0),
                                        op0=ALU.subtract, op1=ALU.is_equal)
                nc.vector.copy_predicated(out=la[0:NH, :N],
                                          mask=hit[32:32 + NH, :N],
                                          data=la[32:32 + NH, :N])
                val2 = epis.tile([NH, NT], F32, tag="val2")
                nc.vector.tensor_tensor(out=val2[:, :N], in0=la[0:NH, :N],
                                        in1=spv_w[:, s0:s0 + N],
                                        op=ALU.subtract)
                nc.vector.tensor_scalar(out=fin_w[:, s0:s0 + N],
                                        in0=spv_w[:, s0:s0 + N],
                                        scalar1=-1.0, scalar2=LOGZERO,
                                        op0=ALU.mult, op1=ALU.add)
                nc.vector.copy_predicated(out=fin_w[:, s0:s0 + N],
                                          mask=sm_w[:, s0:s0 + N],
                                          data=val2[:, :N])
                if c0 == 0:
                    # EOS col: eos - s_prev (BLANK col already LOGZERO - s_prev
                    # since smask[BLANK]=0 is forced host-side)
                    nc.vector.tensor_tensor(out=fin_w[:, EOS:EOS + 1],
                                            in0=eos_sb[:],
                                            in1=spv_w[:, EOS:EOS + 1],
                                            op=ALU.subtract)
            nc.sync.dma_start(out=out_d[:, g0:g0 + W], in_=fin_w[:, :W])

    nc.compile()
    return nc


def make_in_maps(x, r_prev, s_prev, xlens, last_ids, scoring_ids):
    """Per-core input maps: core i owns batch i / hypotheses [8i, 8i+8)."""
    in_maps = []
    for i in range(NCORES):
        hs = slice(i * NH, (i + 1) * NH)
        sids = np.ascontiguousarray(scoring_ids[hs]).astype(np.int64)  # (8,200)
        smask = np.zeros((NH, O), np.int32)
        np.put_along_axis(smask, sids, 1, axis=1)
        smask[:, BLANK] = 0  # BLANK output column is always LOGZERO - s_prev
        in_maps.append({
            "x": np.ascontiguousarray(x[i]).astype(np.float32),
            "rprev": np.ascontiguousarray(r_prev[:, :, hs]).reshape(T, 2 * NH).astype(np.float32),
            "sprev": np.ascontiguousarray(s_prev[hs]).astype(np.float32),
            "lastids": np.ascontiguousarray(last_ids[hs]).astype(np.int32)[:, None],
            "smask": smask,
            "xlen": np.full((128, 1), int(xlens[i]), np.int32),
        })
    return in_maps


_NC_CACHE: dict[int, bass.Bass] = {}


def kernel(x, r_prev, s_prev, xlens, last_ids, scoring_ids, output_length,
           _trace=False):
    x = np.asarray(x)
    r_prev = np.asarray(r_prev)
    s_prev = np.asarray(s_prev)
    xlens = np.asarray(xlens)
    last_ids = np.asarray(last_ids)
    scoring_ids = np.asarray(scoring_ids)
    start = max(int(output_length), 1)
    # output_length == 0 adds an extra x_[0,0] term; inputs here always have
    # output_length >= 1, which this kernel implements.
    assert int(output_length) >= 1, "output_length==0 path not implemented"

    if start not in _NC_CACHE:
        _NC_CACHE[start] = build_nc(start)
    nc = _NC_CACHE[start]

    in_maps = make_in_maps(x, r_prev, s_prev, xlens, last_ids, scoring_ids)
    res = run_bass_kernel_spmd(nc, in_maps, core_ids=list(range(NCORES)),
                               trace=_trace)
    out = np.concatenate([res.results[i]["out"] for i in range(NCORES)], axis=0)
    kernel.last_exec_time_ns = res.exec_time_ns
    kernel.last_results = res
    return out.astype(np.float32)
